# revision 11
# baseline (speedup 1.0000x reference)
"""Trainium2 Bass kernel for nn_Conv2d: x[32,128,56,56] * W[256,128,3,3] + b -> [32,256,56,56].

Stride 1, padding 1, dilation 1. Data-parallel over batch across 8 NeuronCores
(4 images per core, no collectives). Per core the conv is one accumulation
group of 9 matmuls per output tile (one per kernel tap):
PSUM[cout_chunk=128, R*56] += matmul(lhsT=Wt[tap][cin, cout_chunk],
rhs=shifted window of the zero-padded input row-block [cin=128, R+2, 58]).
Bias is fused into the PSUM->SBUF drain on the scalar engine.

Matmuls run in bf16 (1 PE cycle/row vs 4 for exact fp32; enables fast weight
load, so the per-matmul weight switch hides under the previous matmul's
streaming). PSUM accumulation and the output stay fp32; measured absmax rel
err is ~2e-3 vs the fp32 reference. The PSUM output AP is kept in its
natural [128, R, 56] shape — flattening it to [128, 448] measurably slows
every matmul by ~40ns.

DMA flow is just-in-time: x row-tile DMAs are interleaved with the output
DMAs inside the main loop (prefetch depth 5) instead of bulk-issued up
front. The Sync queue triggers DMAs in order through an 8-slot completion
window, so bulk-issuing all 28 input tiles parks every output DMA behind
~8 MB of input traffic -> output SBUF buffers never recycle -> PSUM fills
-> the PE stalls mid-run and the HAM clock-gate re-throttles it (measured
9 us stall + 10 us at half clock). Interleaved issue keeps the PE streaming
continuously.

A 9-matmul warm-up group on zeroed SBUF (result never read) runs during the
initial DMA wait so the HAM activity monitor has the PE at full clock
(2.4 GHz, not the cold 1.2 GHz) before the first real matmul.

Self-contained: hardcodes shapes; host-side pre-pads/retiles x and
pre-transposes W so every device DMA is contiguous.
"""

import numpy as np

B, CIN, H, W_ = 32, 128, 56, 56
COUT, KH, KW = 256, 3, 3
NCORES = 8
BPC = B // NCORES          # images per core
R = 8                      # output rows per tile -> matmul free dim R*56 = 448
NT = H // R                # row tiles per image
NTILE = BPC * NT
HP, WP = H + 2, W_ + 2     # padded 58x58
NCH = COUT // 128          # cout chunks (2)

MM_DTYPE = "bfloat16"
XBUFS = 6                  # x-tile ring depth
PREFETCH = 5               # x tiles loaded ahead of consumption

_cache = {}


def _np_mm_dtype():
    if MM_DTYPE == "bfloat16":
        import ml_dtypes

        return ml_dtypes.bfloat16
    return np.float32


def _build(mm_dtype_name):
    import concourse.mybir as mybir
    import concourse.tile as tile
    from concourse import bacc

    dt = mybir.dt
    mmdt = getattr(dt, mm_dtype_name)

    nc = bacc.Bacc("TRN2", target_bir_lowering=False, debug=False)

    # x arrives host-pre-padded per row-tile: [image, row_tile, cin, R+2, 58]
    # (zero border baked in, halo rows duplicated) so every x DMA is one
    # fully contiguous copy and the kernel needs no memsets.
    x_d = nc.dram_tensor(
        "x", [BPC, NT, CIN, R + 2, WP], mmdt, kind="ExternalInput"
    )
    # [chunk, cin, tap, cout_slice]: one contiguous DMA per cout chunk
    wt_d = nc.dram_tensor("wt", [NCH, CIN, KH * KW, 128], mmdt, kind="ExternalInput")
    b_d = nc.dram_tensor("bias", [128, NCH], dt.float32, kind="ExternalInput")
    # Output laid out [image, cout%128 (partition), cout//128, h, w] so both
    # cout chunks of one row-tile go out in a single DMA; host untangles.
    o_d = nc.dram_tensor(
        "out", [BPC, 128, NCH, H, W_], dt.float32, kind="ExternalOutput"
    )

    with tile.TileContext(nc) as tc:
        with (
            tc.tile_pool(name="const", bufs=1) as const_pool,
            tc.tile_pool(name="xin", bufs=XBUFS) as xin_pool,
            tc.tile_pool(name="outp", bufs=4) as out_pool,
            tc.tile_pool(name="psum", bufs=6, space="PSUM") as psum_pool,
        ):
            xt = []

            def load_x(idx, engine=None):
                n, ht = divmod(idx, NT)
                t = xin_pool.tile([CIN, R + 2, WP], mmdt, tag="xt")
                (engine or nc.sync).dma_start(t[:], x_d[n, ht])
                xt.append(t)

            # PE clock warm-up: the HAM activity monitor keeps the PE at half
            # clock until it has been busy ~3.4us. One 9-matmul group on
            # zeroed SBUF (result never read) during the initial DMA wait
            # brings it to full rate before the first real matmul.
            zw_t = const_pool.tile([CIN, 128], mmdt)
            nc.vector.memset(zw_t[:], 0.0)
            zx_t = const_pool.tile([CIN, R, W_], mmdt)
            nc.vector.memset(zx_t[:], 0.0)
            pw = psum_pool.tile([128, R, W_], dt.float32, tag="ps")
            for i in range(9):
                nc.tensor.matmul(
                    pw[:],
                    zw_t[:],
                    zx_t[:],
                    start=(i == 0),
                    stop=(i == 8),
                )

            # Critical path first: the first x tile (the startup gater) goes
            # out on the otherwise-idle GpSimd queue, concurrent with the
            # weight DMAs on Sync. tap-0 of chunk-0 is split out so the very
            # first matmul only gates on a 32KB transfer.
            load_x(0, engine=nc.gpsimd)
            w_t = const_pool.tile([CIN, NCH, KH * KW, 128], mmdt)
            nc.sync.dma_start(w_t[:, 0, 0], wt_d[0, :, 0])
            nc.sync.dma_start(w_t[:, 0, 1:], wt_d[0, :, 1:])
            nc.sync.dma_start(w_t[:, 1], wt_d[1])
            b_t = const_pool.tile([128, NCH], dt.float32)
            nc.sync.dma_start(b_t[:], b_d[:])
            for i in range(1, PREFETCH):
                load_x(i)

            def mm_group(p, t, c, rows, row0):
                for kh in range(KH):
                    for kw in range(KW):
                        pos = kh * KW + kw
                        nc.tensor.matmul(
                            p[:],
                            w_t[:, c, pos],
                            t[:, row0 + kh : row0 + kh + rows, kw : kw + W_],
                            start=(pos == 0),
                            stop=(pos == KH * KW - 1),
                        )

            def drain(ot, p, c, sl=slice(None)):
                nc.scalar.activation(
                    ot[:, c, sl],
                    p[:],
                    mybir.ActivationFunctionType.Identity,
                    bias=b_t[:, c : c + 1],
                )

            # Tiles 0..25 run as pairs with the 9 weight taps interleaved
            # between the two row-tiles: consecutive matmuls share the same
            # stationary operand, halving weight-load pressure if the
            # backend reuses the loaded array. Tiles 26 and 27 run singly;
            # the final tile's second chunk is split into two half-height
            # groups so its drain + output DMA overlap the last matmuls.
            for m in range(NTILE // 2 - 1):
                i0, i1 = 2 * m, 2 * m + 1
                if i0 + PREFETCH < NTILE:
                    load_x(i0 + PREFETCH)
                if i1 + PREFETCH < NTILE:
                    load_x(i1 + PREFETCH)
                ta, tb = xt[i0], xt[i1]
                ot_a = out_pool.tile([128, NCH, R, W_], dt.float32, tag="ot")
                ot_b = out_pool.tile([128, NCH, R, W_], dt.float32, tag="ot")
                for c in range(NCH):
                    pa = psum_pool.tile([128, R, W_], dt.float32, tag="ps")
                    pb = psum_pool.tile([128, R, W_], dt.float32, tag="ps")
                    for kh in range(KH):
                        for kw in range(KW):
                            pos = kh * KW + kw
                            for p, t in ((pa, ta), (pb, tb)):
                                nc.tensor.matmul(
                                    p[:],
                                    w_t[:, c, pos],
                                    t[:, kh : kh + R, kw : kw + W_],
                                    start=(pos == 0),
                                    stop=(pos == KH * KW - 1),
                                )
                    drain(ot_a, pa, c)
                    drain(ot_b, pb, c)
                for i, ot in ((i0, ot_a), (i1, ot_b)):
                    n, ht = divmod(i, NT)
                    nc.sync.dma_start(
                        o_d[n, :, :, ht * R : ht * R + R, :],
                        ot[:],
                    )

            # tile 26: plain single
            i = NTILE - 2
            n, ht = divmod(i, NT)
            t = xt[i]
            ot = out_pool.tile([128, NCH, R, W_], dt.float32, tag="ot")
            for c in range(NCH):
                p = psum_pool.tile([128, R, W_], dt.float32, tag="ps")
                mm_group(p, t, c, R, 0)
                drain(ot, p, c)
            nc.sync.dma_start(o_d[n, :, :, ht * R : ht * R + R, :], ot[:])

            # tile 27: chunk 0 whole, chunk 1 in two half-height groups so
            # the tail drain and output DMA overlap the final matmuls.
            i = NTILE - 1
            n, ht = divmod(i, NT)
            t = xt[i]
            r0 = ht * R
            ot = out_pool.tile([128, NCH, R, W_], dt.float32, tag="ot")
            p = psum_pool.tile([128, R, W_], dt.float32, tag="ps")
            mm_group(p, t, 0, R, 0)
            drain(ot, p, 0)
            nc.sync.dma_start(o_d[n, :, 0, r0 : r0 + R, :], ot[:, 0])
            for half in range(2):
                hr = R // 2
                ph = psum_pool.tile([128, hr, W_], dt.float32, tag="ph", bufs=2)
                mm_group(ph, t, 1, hr, half * hr)
                sl = slice(half * hr, half * hr + hr)
                drain(ot, ph, 1, sl)
                nc.sync.dma_start(
                    o_d[n, :, 1, r0 + half * hr : r0 + half * hr + hr, :],
                    ot[:, 1, sl],
                )

    nc.compile()
    return nc


def _make_in_maps(x, W, b):
    mdt = _np_mm_dtype()
    x = np.asarray(x, dtype=np.float32)
    W = np.asarray(W, dtype=np.float32)
    b = np.asarray(b, dtype=np.float32)

    # Pre-pad and re-tile x: [B, CIN, 56, 56] -> [B, NT, CIN, R+2, 58] where
    # row-tile ht holds padded rows h0..h0+R+1 (zero border baked in).
    xpad = np.zeros((B, CIN, HP, WP), dtype=mdt)
    xpad[:, :, 1 : H + 1, 1 : W_ + 1] = x.astype(mdt)
    xt = np.empty((B, NT, CIN, R + 2, WP), dtype=mdt)
    for ht in range(NT):
        xt[:, ht] = xpad[:, :, ht * R : ht * R + R + 2, :]

    # [cout, cin, kh, kw] -> [cout_chunk, cin, kh*kw, cout_slice], contiguous
    wt = np.ascontiguousarray(
        W.reshape(NCH, 128, CIN, KH * KW).transpose(0, 2, 3, 1)
    ).astype(mdt)
    bh = np.ascontiguousarray(b.reshape(NCH, 128).T)

    return [
        {
            "x": xt[core * BPC : (core + 1) * BPC],
            "wt": wt,
            "bias": bh,
        }
        for core in range(NCORES)
    ]


def kernel(x, W, b):
    from concourse.bass_utils import run_bass_kernel_spmd

    if MM_DTYPE not in _cache:
        _cache[MM_DTYPE] = _build(MM_DTYPE)
    nc = _cache[MM_DTYPE]

    in_maps = _make_in_maps(x, W, b)
    try:
        res = run_bass_kernel_spmd(nc, in_maps, list(range(NCORES))).results
    except Exception:
        # A prior session can leave the accelerator in a transient
        # unrecoverable state; one retry after re-init clears it.
        import time

        time.sleep(15)
        res = run_bass_kernel_spmd(nc, in_maps, list(range(NCORES))).results
    # [BPC, 128, NCH, H, W] -> [BPC, NCH*128, H, W]
    outs = [
        res[i]["out"].transpose(0, 2, 1, 3, 4).reshape(BPC, COUT, H, W_)
        for i in range(NCORES)
    ]
    return np.concatenate(outs, axis=0)


# revision 12
# speedup vs baseline: 1.1792x; 1.1792x over previous
"""Trainium2 Bass kernel for nn_Conv2d: x[32,128,56,56] * W[256,128,3,3] + b -> [32,256,56,56].

Stride 1, padding 1, dilation 1. Data-parallel over batch across 8 NeuronCores
(4 images per core, no collectives). Per core the conv is one accumulation
group of 9 matmuls per output tile (one per kernel tap):
PSUM[cout_chunk=128, R*56] += matmul(lhsT=Wt[tap][cin, cout_chunk],
rhs=shifted window of the zero-padded input row-block [cin=128, R+2, 58]).
Bias is fused into the PSUM->SBUF drain on the scalar engine.

Matmuls run in bf16 (1 PE cycle/row vs 4 for exact fp32; enables fast weight
load, so the per-matmul weight switch hides under the previous matmul's
streaming). PSUM accumulation and the output stay fp32; measured absmax rel
err is ~2e-3 vs the fp32 reference. The PSUM output AP is kept in its
natural [128, R, 56] shape — flattening it to [128, 448] measurably slows
every matmul by ~40ns.

DMA flow is just-in-time: x row-tile DMAs are interleaved with the output
DMAs inside the main loop (prefetch depth 5) instead of bulk-issued up
front. The Sync queue triggers DMAs in order through an 8-slot completion
window, so bulk-issuing all 28 input tiles parks every output DMA behind
~8 MB of input traffic -> output SBUF buffers never recycle -> PSUM fills
-> the PE stalls mid-run and the HAM clock-gate re-throttles it (measured
9 us stall + 10 us at half clock). Interleaved issue keeps the PE streaming
continuously.

A 9-matmul warm-up group on zeroed SBUF (result never read) runs during the
initial DMA wait so the HAM activity monitor has the PE at full clock
(2.4 GHz, not the cold 1.2 GHz) before the first real matmul.

Self-contained: hardcodes shapes; host-side pre-pads/retiles x and
pre-transposes W so every device DMA is contiguous.
"""

import numpy as np

B, CIN, H, W_ = 32, 128, 56, 56
COUT, KH, KW = 256, 3, 3
NCORES = 8
BPC = B // NCORES          # images per core
R = 8                      # output rows per tile -> matmul free dim R*56 = 448
NT = H // R                # row tiles per image
NTILE = BPC * NT
HP, WP = H + 2, W_ + 2     # padded 58x58
NCH = COUT // 128          # cout chunks (2)

MM_DTYPE = "bfloat16"
XBUFS = 6                  # x-tile ring depth
PREFETCH = 5               # x tiles loaded ahead of consumption

_cache = {}


def _np_mm_dtype():
    if MM_DTYPE == "bfloat16":
        import ml_dtypes

        return ml_dtypes.bfloat16
    return np.float32


def _build(mm_dtype_name):
    import concourse.mybir as mybir
    import concourse.tile as tile
    from concourse import bacc

    dt = mybir.dt
    mmdt = getattr(dt, mm_dtype_name)

    nc = bacc.Bacc("TRN2", target_bir_lowering=False, debug=False)

    # x arrives host-pre-padded per row-tile: [image, row_tile, cin, R+2, 58]
    # (zero border baked in, halo rows duplicated) so every x DMA is one
    # fully contiguous copy and the kernel needs no memsets.
    x_d = nc.dram_tensor(
        "x", [BPC, NT, CIN, R + 2, WP], mmdt, kind="ExternalInput"
    )
    # [chunk, cin, tap, cout_slice]: one contiguous DMA per cout chunk
    wt_d = nc.dram_tensor("wt", [NCH, CIN, KH * KW, 128], mmdt, kind="ExternalInput")
    b_d = nc.dram_tensor("bias", [128, NCH], dt.float32, kind="ExternalInput")
    # Output laid out [image, cout%128 (partition), cout//128, h, w] so both
    # cout chunks of one row-tile go out in a single DMA; host untangles.
    o_d = nc.dram_tensor(
        "out", [BPC, 128, NCH, H, W_], dt.float32, kind="ExternalOutput"
    )

    with tile.TileContext(nc) as tc:
        with (
            tc.tile_pool(name="const", bufs=1) as const_pool,
            tc.tile_pool(name="xin", bufs=XBUFS) as xin_pool,
            tc.tile_pool(name="outp", bufs=4) as out_pool,
            tc.tile_pool(name="psum", bufs=6, space="PSUM") as psum_pool,
        ):
            xt = []

            def load_x(idx, engine=None):
                n, ht = divmod(idx, NT)
                t = xin_pool.tile([CIN, R + 2, WP], mmdt, tag="xt")
                (engine or nc.sync).dma_start(t[:], x_d[n, ht])
                xt.append(t)

            # PE clock warm-up: the HAM activity monitor keeps the PE at half
            # clock until it has been busy ~3.4us. One 9-matmul group on
            # zeroed SBUF (result never read) during the initial DMA wait
            # brings it to full rate before the first real matmul.
            zw_t = const_pool.tile([CIN, 128], mmdt)
            nc.vector.memset(zw_t[:], 0.0)
            zx_t = const_pool.tile([CIN, R, W_], mmdt)
            nc.vector.memset(zx_t[:], 0.0)
            pw = psum_pool.tile([128, R, W_], dt.float32, tag="ps")
            for i in range(9):
                nc.tensor.matmul(
                    pw[:],
                    zw_t[:],
                    zx_t[:],
                    start=(i == 0),
                    stop=(i == 8),
                )

            # Critical path first: the first x tile (the startup gater) goes
            # out on the otherwise-idle GpSimd queue, concurrent with the
            # weight DMAs on Sync. tap-0 of chunk-0 is split out so the very
            # first matmul only gates on a 32KB transfer.
            load_x(0, engine=nc.gpsimd)
            w_t = const_pool.tile([CIN, NCH, KH * KW, 128], mmdt)
            nc.sync.dma_start(w_t[:, 0, 0], wt_d[0, :, 0])
            nc.sync.dma_start(w_t[:, 0, 1:], wt_d[0, :, 1:])
            nc.sync.dma_start(w_t[:, 1], wt_d[1])
            b_t = const_pool.tile([128, NCH], dt.float32)
            nc.sync.dma_start(b_t[:], b_d[:])
            for i in range(1, PREFETCH):
                load_x(i)

            def mm_group(p, t, c, rows, row0):
                for kh in range(KH):
                    for kw in range(KW):
                        pos = kh * KW + kw
                        nc.tensor.matmul(
                            p[:],
                            w_t[:, c, pos],
                            t[:, row0 + kh : row0 + kh + rows, kw : kw + W_],
                            start=(pos == 0),
                            stop=(pos == KH * KW - 1),
                        )

            def drain(ot, p, c, sl=slice(None)):
                nc.scalar.activation(
                    ot[:, c, sl],
                    p[:],
                    mybir.ActivationFunctionType.Identity,
                    bias=b_t[:, c : c + 1],
                )

            # Tiles 0..26: one 9-matmul accumulation group per cout chunk.
            # Keep each group's matmuls consecutive on the same PSUM bank
            # with the same rhs tile — any per-matmul alternation of PSUM
            # target or flattened output AP measurably costs ~40ns/matmul.
            for idx in range(NTILE - 1):
                n, ht = divmod(idx, NT)
                if idx + PREFETCH < NTILE:
                    load_x(idx + PREFETCH)
                t = xt[idx]
                ot = out_pool.tile([128, NCH, R, W_], dt.float32, tag="ot")
                for c in range(NCH):
                    p = psum_pool.tile([128, R, W_], dt.float32, tag="ps")
                    mm_group(p, t, c, R, 0)
                    drain(ot, p, c)
                nc.sync.dma_start(
                    o_d[n, :, :, ht * R : ht * R + R, :],
                    ot[:],
                )

            # tile 27: chunk 0 whole, chunk 1 in two half-height groups so
            # the tail drain and output DMA overlap the final matmuls.
            i = NTILE - 1
            n, ht = divmod(i, NT)
            t = xt[i]
            r0 = ht * R
            ot = out_pool.tile([128, NCH, R, W_], dt.float32, tag="ot")
            p = psum_pool.tile([128, R, W_], dt.float32, tag="ps")
            mm_group(p, t, 0, R, 0)
            drain(ot, p, 0)
            nc.sync.dma_start(o_d[n, :, 0, r0 : r0 + R, :], ot[:, 0])
            for half in range(2):
                hr = R // 2
                ph = psum_pool.tile([128, hr, W_], dt.float32, tag="ph", bufs=2)
                mm_group(ph, t, 1, hr, half * hr)
                sl = slice(half * hr, half * hr + hr)
                drain(ot, ph, 1, sl)
                nc.sync.dma_start(
                    o_d[n, :, 1, r0 + half * hr : r0 + half * hr + hr, :],
                    ot[:, 1, sl],
                )

    nc.compile()
    return nc


def _make_in_maps(x, W, b):
    mdt = _np_mm_dtype()
    x = np.asarray(x, dtype=np.float32)
    W = np.asarray(W, dtype=np.float32)
    b = np.asarray(b, dtype=np.float32)

    # Pre-pad and re-tile x: [B, CIN, 56, 56] -> [B, NT, CIN, R+2, 58] where
    # row-tile ht holds padded rows h0..h0+R+1 (zero border baked in).
    xpad = np.zeros((B, CIN, HP, WP), dtype=mdt)
    xpad[:, :, 1 : H + 1, 1 : W_ + 1] = x.astype(mdt)
    xt = np.empty((B, NT, CIN, R + 2, WP), dtype=mdt)
    for ht in range(NT):
        xt[:, ht] = xpad[:, :, ht * R : ht * R + R + 2, :]

    # [cout, cin, kh, kw] -> [cout_chunk, cin, kh*kw, cout_slice], contiguous
    wt = np.ascontiguousarray(
        W.reshape(NCH, 128, CIN, KH * KW).transpose(0, 2, 3, 1)
    ).astype(mdt)
    bh = np.ascontiguousarray(b.reshape(NCH, 128).T)

    return [
        {
            "x": xt[core * BPC : (core + 1) * BPC],
            "wt": wt,
            "bias": bh,
        }
        for core in range(NCORES)
    ]


def kernel(x, W, b):
    from concourse.bass_utils import run_bass_kernel_spmd

    if MM_DTYPE not in _cache:
        _cache[MM_DTYPE] = _build(MM_DTYPE)
    nc = _cache[MM_DTYPE]

    in_maps = _make_in_maps(x, W, b)
    try:
        res = run_bass_kernel_spmd(nc, in_maps, list(range(NCORES))).results
    except Exception:
        # A prior session can leave the accelerator in a transient
        # unrecoverable state; one retry after re-init clears it.
        import time

        time.sleep(15)
        res = run_bass_kernel_spmd(nc, in_maps, list(range(NCORES))).results
    # [BPC, 128, NCH, H, W] -> [BPC, NCH*128, H, W]
    outs = [
        res[i]["out"].transpose(0, 2, 1, 3, 4).reshape(BPC, COUT, H, W_)
        for i in range(NCORES)
    ]
    return np.concatenate(outs, axis=0)


# revision 16
# speedup vs baseline: 1.1799x; 1.0006x over previous
"""Trainium2 Bass kernel for nn_Conv2d: x[32,128,56,56] * W[256,128,3,3] + b -> [32,256,56,56].

Stride 1, padding 1, dilation 1. Data-parallel over batch across 8 NeuronCores
(4 images per core, no collectives). Per core the conv is one accumulation
group of 9 matmuls per output tile (one per kernel tap):
PSUM[cout_chunk=128, R*56] += matmul(lhsT=Wt[tap][cin, cout_chunk],
rhs=shifted window of the zero-padded input row-block [cin=128, R+2, 58]).
Bias is fused into the PSUM->SBUF drain on the scalar engine.

Matmuls run in bf16 (1 PE cycle/row vs 4 for exact fp32; enables fast weight
load, so the per-matmul weight switch hides under the previous matmul's
streaming). PSUM accumulation and the output stay fp32; measured absmax rel
err is ~2e-3 vs the fp32 reference. The PSUM output AP is kept in its
natural [128, R, 56] shape — flattening it to [128, 448] measurably slows
every matmul by ~40ns.

DMA flow is just-in-time: x row-tile DMAs are interleaved with the output
DMAs inside the main loop (prefetch depth 5) instead of bulk-issued up
front. The Sync queue triggers DMAs in order through an 8-slot completion
window, so bulk-issuing all 28 input tiles parks every output DMA behind
~8 MB of input traffic -> output SBUF buffers never recycle -> PSUM fills
-> the PE stalls mid-run and the HAM clock-gate re-throttles it (measured
9 us stall + 10 us at half clock). Interleaved issue keeps the PE streaming
continuously.

A 9-matmul warm-up group on zeroed SBUF (result never read) runs during the
initial DMA wait so the HAM activity monitor has the PE at full clock
(2.4 GHz, not the cold 1.2 GHz) before the first real matmul.

Self-contained: hardcodes shapes; host-side pre-pads/retiles x and
pre-transposes W so every device DMA is contiguous.
"""

import numpy as np

B, CIN, H, W_ = 32, 128, 56, 56
COUT, KH, KW = 256, 3, 3
NCORES = 8
BPC = B // NCORES          # images per core
R = 8                      # output rows per tile -> matmul free dim R*56 = 448
NT = H // R                # row tiles per image
NTILE = BPC * NT
HP, WP = H + 2, W_ + 2     # padded 58x58
NCH = COUT // 128          # cout chunks (2)

MM_DTYPE = "bfloat16"
XBUFS = 6                  # x-tile ring depth
PREFETCH = 5               # x tiles loaded ahead of consumption

_cache = {}


def _np_mm_dtype():
    if MM_DTYPE == "bfloat16":
        import ml_dtypes

        return ml_dtypes.bfloat16
    return np.float32


def _build(mm_dtype_name):
    import concourse.mybir as mybir
    import concourse.tile as tile
    from concourse import bacc

    dt = mybir.dt
    mmdt = getattr(dt, mm_dtype_name)

    nc = bacc.Bacc("TRN2", target_bir_lowering=False, debug=False)

    # x arrives host-pre-padded per row-tile: [image, row_tile, cin, R+2, 58]
    # (zero border baked in, halo rows duplicated) so every x DMA is one
    # fully contiguous copy and the kernel needs no memsets.
    x_d = nc.dram_tensor(
        "x", [BPC, NT, CIN, R + 2, WP], mmdt, kind="ExternalInput"
    )
    # [chunk, cin, tap, cout_slice]: one contiguous DMA per cout chunk
    wt_d = nc.dram_tensor("wt", [NCH, CIN, KH * KW, 128], mmdt, kind="ExternalInput")
    b_d = nc.dram_tensor("bias", [128, NCH], dt.float32, kind="ExternalInput")
    # Output laid out [image, cout%128 (partition), cout//128, h, w] so both
    # cout chunks of one row-tile go out in a single DMA; host untangles.
    o_d = nc.dram_tensor(
        "out", [BPC, 128, NCH, H, W_], dt.float32, kind="ExternalOutput"
    )

    with tile.TileContext(nc) as tc:
        with (
            tc.tile_pool(name="const", bufs=1) as const_pool,
            tc.tile_pool(name="xin", bufs=XBUFS) as xin_pool,
            tc.tile_pool(name="outp", bufs=4) as out_pool,
            tc.tile_pool(name="psum", bufs=6, space="PSUM") as psum_pool,
        ):
            xt = []

            def load_x(idx, engine=None):
                n, ht = divmod(idx, NT)
                t = xin_pool.tile([CIN, R + 2, WP], mmdt, tag="xt")
                (engine or nc.sync).dma_start(t[:], x_d[n, ht])
                xt.append(t)

            # The first x tile (the startup gater) goes out on the
            # otherwise-idle GpSimd queue ahead of everything else there,
            # concurrent with the weight DMAs on Sync.
            load_x(0, engine=nc.gpsimd)

            # PE clock warm-up: the HAM activity monitor keeps the PE at half
            # clock until one full (free-running) ~3.4us activity window has
            # seen it busy. A 10-matmul group (~3.7us at the cold rate) on
            # zeroed SBUF (result never read) during the initial DMA wait
            # makes the flip to full rate land before or shortly after the
            # first real matmul. (The operands must be initialized — the
            # build-time shadow simulator rejects uninitialized reads.)
            zw_t = const_pool.tile([CIN, 128], mmdt)
            nc.gpsimd.memset(zw_t[:], 0.0)
            zx_t = const_pool.tile([CIN, R, W_], mmdt)
            nc.gpsimd.memset(zx_t[:], 0.0)
            pw = psum_pool.tile([128, R, W_], dt.float32, tag="ps")
            for i in range(10):
                nc.tensor.matmul(
                    pw[:],
                    zw_t[:],
                    zx_t[:],
                    start=(i == 0),
                    stop=(i == 9),
                )

            # tap-0 of chunk-0 is split out so the very first matmul only
            # gates on a 32KB transfer.
            w_t = const_pool.tile([CIN, NCH, KH * KW, 128], mmdt)
            nc.sync.dma_start(w_t[:, 0, 0], wt_d[0, :, 0])
            nc.sync.dma_start(w_t[:, 0, 1:], wt_d[0, :, 1:])
            nc.sync.dma_start(w_t[:, 1], wt_d[1])
            b_t = const_pool.tile([128, NCH], dt.float32)
            nc.sync.dma_start(b_t[:], b_d[:])
            for i in range(1, PREFETCH):
                load_x(i)

            def mm_group(p, t, c, rows, row0):
                for kh in range(KH):
                    for kw in range(KW):
                        pos = kh * KW + kw
                        nc.tensor.matmul(
                            p[:],
                            w_t[:, c, pos],
                            t[:, row0 + kh : row0 + kh + rows, kw : kw + W_],
                            start=(pos == 0),
                            stop=(pos == KH * KW - 1),
                        )

            def drain(ot, p, c, sl=slice(None)):
                nc.scalar.activation(
                    ot[:, c, sl],
                    p[:],
                    mybir.ActivationFunctionType.Identity,
                    bias=b_t[:, c : c + 1],
                )

            # Tiles 0..26: one 9-matmul accumulation group per cout chunk.
            # Keep each group's matmuls consecutive on the same PSUM bank
            # with the same rhs tile — any per-matmul alternation of PSUM
            # target or flattened output AP measurably costs ~40ns/matmul.
            for idx in range(NTILE - 1):
                n, ht = divmod(idx, NT)
                if idx + PREFETCH < NTILE:
                    load_x(idx + PREFETCH)
                t = xt[idx]
                ot = out_pool.tile([128, NCH, R, W_], dt.float32, tag="ot")
                for c in range(NCH):
                    p = psum_pool.tile([128, R, W_], dt.float32, tag="ps")
                    mm_group(p, t, c, R, 0)
                    drain(ot, p, c)
                nc.sync.dma_start(
                    o_d[n, :, :, ht * R : ht * R + R, :],
                    ot[:],
                )

            # tile 27: chunk 0 whole, chunk 1 in two half-height groups so
            # the tail drain and output DMA overlap the final matmuls.
            i = NTILE - 1
            n, ht = divmod(i, NT)
            t = xt[i]
            r0 = ht * R
            ot = out_pool.tile([128, NCH, R, W_], dt.float32, tag="ot")
            p = psum_pool.tile([128, R, W_], dt.float32, tag="ps")
            mm_group(p, t, 0, R, 0)
            drain(ot, p, 0)
            nc.sync.dma_start(o_d[n, :, 0, r0 : r0 + R, :], ot[:, 0])
            for half in range(2):
                hr = R // 2
                ph = psum_pool.tile([128, hr, W_], dt.float32, tag="ph", bufs=2)
                mm_group(ph, t, 1, hr, half * hr)
                sl = slice(half * hr, half * hr + hr)
                drain(ot, ph, 1, sl)
                nc.sync.dma_start(
                    o_d[n, :, 1, r0 + half * hr : r0 + half * hr + hr, :],
                    ot[:, 1, sl],
                )

    nc.compile()
    return nc


def _make_in_maps(x, W, b):
    mdt = _np_mm_dtype()
    x = np.asarray(x, dtype=np.float32)
    W = np.asarray(W, dtype=np.float32)
    b = np.asarray(b, dtype=np.float32)

    # Pre-pad and re-tile x: [B, CIN, 56, 56] -> [B, NT, CIN, R+2, 58] where
    # row-tile ht holds padded rows h0..h0+R+1 (zero border baked in).
    xpad = np.zeros((B, CIN, HP, WP), dtype=mdt)
    xpad[:, :, 1 : H + 1, 1 : W_ + 1] = x.astype(mdt)
    xt = np.empty((B, NT, CIN, R + 2, WP), dtype=mdt)
    for ht in range(NT):
        xt[:, ht] = xpad[:, :, ht * R : ht * R + R + 2, :]

    # [cout, cin, kh, kw] -> [cout_chunk, cin, kh*kw, cout_slice], contiguous
    wt = np.ascontiguousarray(
        W.reshape(NCH, 128, CIN, KH * KW).transpose(0, 2, 3, 1)
    ).astype(mdt)
    bh = np.ascontiguousarray(b.reshape(NCH, 128).T)

    return [
        {
            "x": xt[core * BPC : (core + 1) * BPC],
            "wt": wt,
            "bias": bh,
        }
        for core in range(NCORES)
    ]


def kernel(x, W, b):
    from concourse.bass_utils import run_bass_kernel_spmd

    if MM_DTYPE not in _cache:
        _cache[MM_DTYPE] = _build(MM_DTYPE)
    nc = _cache[MM_DTYPE]

    in_maps = _make_in_maps(x, W, b)
    try:
        res = run_bass_kernel_spmd(nc, in_maps, list(range(NCORES))).results
    except Exception:
        # A prior session can leave the accelerator in a transient
        # unrecoverable state; one retry after re-init clears it.
        import time

        time.sleep(15)
        res = run_bass_kernel_spmd(nc, in_maps, list(range(NCORES))).results
    # [BPC, 128, NCH, H, W] -> [BPC, NCH*128, H, W]
    outs = [
        res[i]["out"].transpose(0, 2, 1, 3, 4).reshape(BPC, COUT, H, W_)
        for i in range(NCORES)
    ]
    return np.concatenate(outs, axis=0)


# revision 17
# speedup vs baseline: 1.1895x; 1.0082x over previous
"""Trainium2 Bass kernel for nn_Conv2d: x[32,128,56,56] * W[256,128,3,3] + b -> [32,256,56,56].

Stride 1, padding 1, dilation 1. Data-parallel over batch across 8 NeuronCores
(4 images per core, no collectives). Per core the conv is one accumulation
group of 9 matmuls per output tile (one per kernel tap):
PSUM[cout_chunk=128, R*56] += matmul(lhsT=Wt[tap][cin, cout_chunk],
rhs=shifted window of the zero-padded input row-block [cin=128, R+2, 58]).
Bias is fused into the PSUM->SBUF drain on the scalar engine.

Matmuls run in bf16 (1 PE cycle/row vs 4 for exact fp32; enables fast weight
load, so the per-matmul weight switch hides under the previous matmul's
streaming). PSUM accumulation and the output stay fp32; measured absmax rel
err is ~2e-3 vs the fp32 reference (tolerance 2e-2).

Hard-won scheduling facts baked in here (each measured on hardware):
- Each group's 9 matmuls must stay consecutive, on one PSUM bank, with the
  natural [128, R, 56] output AP. Flattening the AP to [128, 448] or
  alternating PSUM banks/rhs tiles between matmuls costs ~40ns on every
  matmul (~20% of its 191ns pitch).
- x row-tile DMAs are issued just-in-time inside the main loop (prefetch
  depth 5), interleaved with output DMAs. The Sync queue triggers DMAs in
  order through an 8-slot completion window, so bulk-issuing all 28 input
  tiles up front parks every output DMA behind ~8MB of input traffic ->
  output SBUF buffers never recycle -> PSUM fills -> the PE stalls mid-run
  and the HAM clock gate re-throttles it (measured 9us stall + 10us at half
  clock).
- A 10-matmul warm-up group on zeroed SBUF (result never read) runs during
  the initial DMA wait so the HAM activity monitor flips the PE to full
  clock (2.4GHz, not the cold 1.2GHz) before/just after the first real
  matmul.
- The final tile ships one DMA per cout chunk so the last transfer is half
  as deep.
- The framework epilogue serially resets every allocated semaphore
  (~115ns each, ~8.7us total) — every extra pool tag, DMA, or engine queue
  used inflates it, which is why this kernel keeps the structure minimal
  (measured: fancier variants gained ~1.4us in flight but paid it all back
  in epilogue).

Self-contained: hardcodes shapes; host-side pre-pads/retiles x and
pre-transposes W so every device DMA is contiguous.
"""

import numpy as np

B, CIN, H, W_ = 32, 128, 56, 56
COUT, KH, KW = 256, 3, 3
NCORES = 8
BPC = B // NCORES          # images per core
R = 8                      # output rows per tile -> matmul free dim R*56 = 448
NT = H // R                # row tiles per image
NTILE = BPC * NT
HP, WP = H + 2, W_ + 2     # padded 58x58
NCH = COUT // 128          # cout chunks (2)

MM_DTYPE = "bfloat16"
XBUFS = 6                  # x-tile ring depth
PREFETCH = 5               # x tiles loaded ahead of consumption

_cache = {}


def _np_mm_dtype():
    if MM_DTYPE == "bfloat16":
        import ml_dtypes

        return ml_dtypes.bfloat16
    return np.float32


def _build(mm_dtype_name):
    import concourse.mybir as mybir
    import concourse.tile as tile
    from concourse import bacc

    dt = mybir.dt
    mmdt = getattr(dt, mm_dtype_name)

    nc = bacc.Bacc("TRN2", target_bir_lowering=False, debug=False)

    # x arrives host-pre-padded per row-tile: [image, row_tile, cin, R+2, 58]
    # (zero border baked in, halo rows duplicated) so every x DMA is one
    # fully contiguous copy and the kernel needs no memsets.
    x_d = nc.dram_tensor(
        "x", [BPC, NT, CIN, R + 2, WP], mmdt, kind="ExternalInput"
    )
    # [chunk, cin, tap, cout_slice]: one contiguous DMA per cout chunk
    wt_d = nc.dram_tensor("wt", [NCH, CIN, KH * KW, 128], mmdt, kind="ExternalInput")
    b_d = nc.dram_tensor("bias", [128, NCH], dt.float32, kind="ExternalInput")
    # Output laid out [image, cout%128 (partition), cout//128, h, w] so both
    # cout chunks of one row-tile go out in a single DMA; host untangles.
    o_d = nc.dram_tensor(
        "out", [BPC, 128, NCH, H, W_], dt.float32, kind="ExternalOutput"
    )

    with tile.TileContext(nc) as tc:
        with (
            tc.tile_pool(name="const", bufs=1) as const_pool,
            tc.tile_pool(name="xin", bufs=XBUFS) as xin_pool,
            tc.tile_pool(name="outp", bufs=4) as out_pool,
            tc.tile_pool(name="psum", bufs=8, space="PSUM") as psum_pool,
        ):
            xt = []

            def load_x(idx):
                n, ht = divmod(idx, NT)
                t = xin_pool.tile([CIN, R + 2, WP], mmdt, tag="xt")
                nc.sync.dma_start(t[:], x_d[n, ht])
                xt.append(t)

            # PE clock warm-up (see module docstring).
            zw_t = const_pool.tile([CIN, 128], mmdt)
            nc.gpsimd.memset(zw_t[:], 0.0)
            zx_t = const_pool.tile([CIN, R, W_], mmdt)
            nc.gpsimd.memset(zx_t[:], 0.0)
            pw = psum_pool.tile([128, R, W_], dt.float32, tag="ps")
            for i in range(10):
                nc.tensor.matmul(
                    pw[:],
                    zw_t[:],
                    zx_t[:],
                    start=(i == 0),
                    stop=(i == 9),
                )

            # Critical path first: the first x tile (the startup gater), then
            # tap-0 of chunk-0 weights (all the first matmul needs), then the
            # rest of the constants and the prefetch window.
            load_x(0)
            w_t = const_pool.tile([CIN, NCH, KH * KW, 128], mmdt)
            nc.sync.dma_start(w_t[:, 0, 0], wt_d[0, :, 0])
            nc.sync.dma_start(w_t[:, 0, 1:], wt_d[0, :, 1:])
            nc.sync.dma_start(w_t[:, 1], wt_d[1])
            b_t = const_pool.tile([128, NCH], dt.float32)
            nc.sync.dma_start(b_t[:], b_d[:])
            for i in range(1, PREFETCH):
                load_x(i)

            for idx in range(NTILE):
                n, ht = divmod(idx, NT)
                if idx + PREFETCH < NTILE:
                    load_x(idx + PREFETCH)
                t = xt[idx]
                ot = out_pool.tile([128, NCH, R, W_], dt.float32, tag="ot")
                for c in range(NCH):
                    p = psum_pool.tile([128, R, W_], dt.float32, tag="ps")
                    for kh in range(KH):
                        for kw in range(KW):
                            pos = kh * KW + kw
                            nc.tensor.matmul(
                                p[:],
                                w_t[:, c, pos],
                                t[:, kh : kh + R, kw : kw + W_],
                                start=(pos == 0),
                                stop=(pos == KH * KW - 1),
                            )
                    nc.scalar.activation(
                        ot[:, c],
                        p[:],
                        mybir.ActivationFunctionType.Identity,
                        bias=b_t[:, c : c + 1],
                    )
                    if idx == NTILE - 1:
                        # Tail latency: ship each chunk of the final tile as
                        # soon as its drain finishes instead of waiting for
                        # both.
                        nc.sync.dma_start(
                            o_d[n, :, c, ht * R : ht * R + R, :],
                            ot[:, c],
                        )
                if idx < NTILE - 1:
                    nc.sync.dma_start(
                        o_d[n, :, :, ht * R : ht * R + R, :],
                        ot[:],
                    )

    nc.compile()
    return nc


def _make_in_maps(x, W, b):
    mdt = _np_mm_dtype()
    x = np.asarray(x, dtype=np.float32)
    W = np.asarray(W, dtype=np.float32)
    b = np.asarray(b, dtype=np.float32)

    # Pre-pad and re-tile x: [B, CIN, 56, 56] -> [B, NT, CIN, R+2, 58] where
    # row-tile ht holds padded rows h0..h0+R+1 (zero border baked in).
    xpad = np.zeros((B, CIN, HP, WP), dtype=mdt)
    xpad[:, :, 1 : H + 1, 1 : W_ + 1] = x.astype(mdt)
    xt = np.empty((B, NT, CIN, R + 2, WP), dtype=mdt)
    for ht in range(NT):
        xt[:, ht] = xpad[:, :, ht * R : ht * R + R + 2, :]

    # [cout, cin, kh, kw] -> [cout_chunk, cin, kh*kw, cout_slice], contiguous
    wt = np.ascontiguousarray(
        W.reshape(NCH, 128, CIN, KH * KW).transpose(0, 2, 3, 1)
    ).astype(mdt)
    bh = np.ascontiguousarray(b.reshape(NCH, 128).T)

    return [
        {
            "x": xt[core * BPC : (core + 1) * BPC],
            "wt": wt,
            "bias": bh,
        }
        for core in range(NCORES)
    ]


def kernel(x, W, b):
    from concourse.bass_utils import run_bass_kernel_spmd

    if MM_DTYPE not in _cache:
        _cache[MM_DTYPE] = _build(MM_DTYPE)
    nc = _cache[MM_DTYPE]

    in_maps = _make_in_maps(x, W, b)
    try:
        res = run_bass_kernel_spmd(nc, in_maps, list(range(NCORES))).results
    except Exception:
        # A prior session can leave the accelerator in a transient
        # unrecoverable state; one retry after re-init clears it.
        import time

        time.sleep(15)
        res = run_bass_kernel_spmd(nc, in_maps, list(range(NCORES))).results
    # [BPC, 128, NCH, H, W] -> [BPC, NCH*128, H, W]
    outs = [
        res[i]["out"].transpose(0, 2, 1, 3, 4).reshape(BPC, COUT, H, W_)
    for i in range(NCORES)
    ]
    return np.concatenate(outs, axis=0)
